# revision 43
# baseline (speedup 1.0000x reference)
"""Distributed Trainium2 attention kernel (8 NeuronCores, batch x head-group parallel).

Reference: y = Attention(x) with RoPE + causal mask, B=2, L=2048, D=2048, H=16, DH=128.

Sharding: 2 batches x 4 head-groups -> core c handles batch c//4, heads
[4*(c%4), 4*(c%4)+4). Each core computes its 4 heads end-to-end over its
batch's 2048 tokens and a full-width partial of the output projection
y_c = out_c @ wo_c^T; the host sums the 4 partials per batch (row-parallel
reduce), so no on-device collective is needed. vs pure head sharding this
halves per-core x/y traffic and doubles the v-projection matmul width.

Layout (all host-side prep, no on-device transposes):
  - x is pre-transposed per batch to xT [D, L] so QKV projections contract
    along the partition dim and produce qT/kT [DH, L] per head.
  - v is produced in natural [token, feat] layout from the same xT panels
    (operand-swapped matmul), free dim E=512.
  - scores are computed TRANSPOSED: sT[k,q] = kT.T @ qT; softmax runs along
    the partition (k) axis: exp on ACT (no max-subtraction; |s| < ~10),
    denominator via ones-vector matmul accumulated in PSUM, reciprocal on
    DVE, broadcast back via a rank-1 matmul.
  - causal mask is handled by loop bounds; the 4 diagonal-straddling k-tiles
    per q-chunk get -1e9 added IN PSUM via one extra matmul
    (identity @ precomputed mask rows) so exp underflows to exact 0 -- no
    extra DVE pass and no PE->ACT->DVE->PE round trip.
  - RoPE: q' = qa*C + (A@qa)*S where A is the constant pair-swap/negate
    matrix (one extra 128x128 matmul per tile).
All PE inputs are bf16 (fp32 PSUM accumulation). All matmuls use N=512.
"""

import numpy as np
import ml_dtypes

import concourse.bass as bass
import concourse.mybir as mybir
from concourse import tile
from concourse.bass_utils import run_bass_kernel_spmd

B, L, D, H = 2, 2048, 2048, 16
DH = D // H          # 128
NCORES = 8
GROUPS = 4           # head-groups per batch
HPC = H // GROUPS    # 4 heads per core
E = HPC * DH         # 512 local features
KT = D // 128        # 16 contraction tiles
QC = 512             # matmul free-dim / q-chunk width
NCH = L // QC        # 4 token chunks per core
TT = L // 128        # 16 token tiles per core
BF = mybir.dt.bfloat16
F32 = mybir.dt.float32
ISCALE = 1.0 / np.sqrt(DH)
NEG = -1e9


def _split_multi_waits(raw: bytes) -> bytes:
    """Walrus on this toolchain rejects instructions carrying 2+ sync waits
    (fixed-capacity sync slots in the ISA structs). Hoist all but one wait of
    every instruction onto standalone single-wait EventSemaphore ops placed
    immediately before it in the same engine's stream (identical blocking
    semantics -- the engine stalls at the EventSemaphore instead)."""
    import orjson
    d = orjson.loads(raw)
    ctr = [0]

    def fix(o):
        if isinstance(o, dict):
            insts = o.get("instructions")
            if isinstance(insts, list) and insts and isinstance(insts[0], dict) \
                    and "opcode" in insts[0]:
                out = []
                for inst in insts:
                    si = inst.get("sync_info")
                    ws = (si or {}).get("on_wait") or []
                    if len(ws) >= 2 and inst.get("opcode") != "EventSemaphore":
                        for w in ws[:-1]:
                            ctr[0] += 1
                            out.append({"debug": inst.get("debug", 0),
                                        "engine": inst["engine"], "ins": [],
                                        "name": f"WS-{ctr[0]}",
                                        "opcode": "EventSemaphore", "outs": [],
                                        "sync_info": {"on_update": [],
                                                      "on_wait": [w]}})
                        si["on_wait"] = [ws[-1]]
                    out.append(inst)
                o["instructions"] = out
            for v in o.values():
                fix(v)
        elif isinstance(o, list):
            for x in o:
                fix(x)

    fix(d)
    return orjson.dumps(d)


import concourse.bass2jax as _b2j

_orig_decompress = _b2j._decompress_ant_bir


def _patched_decompress(v):
    return _split_multi_waits(_orig_decompress(v))


_b2j._decompress_ant_bir = _patched_decompress


def build_nc():
    nc = bass.Bass("TRN2", target_bir_lowering=False)

    # All big operands arrive HOST-PRE-PERMUTED to partition-major layouts so
    # every DMA is a contiguous line-rate copy (the strided gather views cost
    # ~3x in DMA time and ~10us of startup latency).
    # xP[p, (c*KT + t)*QC + q] = x[c*QC + q, t*128 + p]  (chunk-tile-major, so
    # both the chunk DMA and the SBUF matmul views are contiguous)
    xP = nc.declare_dram_parameter("xP", [128, L * KT], BF, isOutput=False)
    # w*P[p, t*E + e] = w*T[t*128 + p, e]
    wq = nc.declare_dram_parameter("wqP", [128, KT * E], BF, isOutput=False)
    wk = nc.declare_dram_parameter("wkP", [128, KT * E], BF, isOutput=False)
    wv = nc.declare_dram_parameter("wvP", [128, KT * E], BF, isOutput=False)
    # woP[p, h*D + d] = woT[h*128 + p, d]
    wo = nc.declare_dram_parameter("woP", [128, HPC * D], BF, isOutput=False)
    Ct = nc.declare_dram_parameter("Ct", [DH, L], BF, isOutput=False)
    St = nc.declare_dram_parameter("St", [DH, L], BF, isOutput=False)
    At = nc.declare_dram_parameter("At", [DH, DH], BF, isOutput=False)
    eye = nc.declare_dram_parameter("eye", [DH, DH], BF, isOutput=False)
    ones = nc.declare_dram_parameter("ones", [DH, 1], BF, isOutput=False)
    onesr = nc.declare_dram_parameter("onesr", [1, DH], BF, isOutput=False)
    negm = nc.declare_dram_parameter("negm", [128, 128], BF, isOutput=False)
    y = nc.declare_dram_parameter("y", [L, D], BF, isOutput=True)

    with tile.TileContext(nc) as tc:
        with (
            tc.tile_pool(name="const", bufs=1) as cpool,
            tc.tile_pool(name="qkv", bufs=1) as qkvpool,
            tc.tile_pool(name="xin", bufs=2) as xpool,
            tc.tile_pool(name="work", bufs=4) as wpool,
            tc.tile_pool(name="exw", bufs=4) as expool,
            tc.tile_pool(name="ysb", bufs=2) as ypool,
        ):
            # ---- resident constants ----
            # Startup is DMA-latency bound: split the weight streams across
            # BOTH HWDGE rings so the q-path (wq on ACT ring) and the x chunk
            # + wk (SP ring) arrive in parallel. wv/wo/small consts follow.
            wq_sb = cpool.tile([128, KT * E], BF, tag="wq")
            wk_sb = cpool.tile([128, KT * E], BF, tag="wk")
            wv_sb = cpool.tile([128, KT * E], BF, tag="wv")
            A_sb = cpool.tile([128, 128], BF, tag="A")
            C_sb = cpool.tile([128, L], BF, tag="C")
            S_sb = cpool.tile([128, L], BF, tag="S")
            # split loads: compute on early k-tiles starts while the rest stream
            def wload(eng, wsb, wdram, splits):
                t0 = 0
                for hk in splits:
                    eng.dma_start(
                        out=wsb[:, t0 * E:(t0 + hk) * E],
                        in_=wdram[:, t0 * E:(t0 + hk) * E])
                    t0 += hk

            # All weight/const loads ride the GPSIMD SWDGE ring: the ACT and
            # SP engines stay free of descriptor generation (an ACT-issued
            # DMA occupies the ACT instruction stream for ~1-6us, delaying
            # activations queued behind it).
            wload(nc.gpsimd, wq_sb, wq, [2, 2, 4, 8])
            # Serialize the rest of the SWDGE stream behind wq: this gpsimd
            # read of wq_sb makes every later gpsimd DMA issue wait until wq
            # has landed, so wq + the first x chunk get the full HBM
            # bandwidth instead of sharing it with 6 other streams.
            wqgate = wpool.tile([1, 16], BF, tag="wqgate", bufs=1)
            nc.gpsimd.tensor_copy(wqgate[:], wq_sb[0:1, KT * E - 16:])
            nc.gpsimd.dma_start(out=A_sb[:], in_=At[:, :])
            nc.gpsimd.dma_start(out=C_sb[:], in_=Ct[:, :])
            nc.gpsimd.dma_start(out=S_sb[:], in_=St[:, :])
            wload(nc.gpsimd, wk_sb, wk, [8, 8])
            wload(nc.gpsimd, wv_sb, wv, [8, 8])
            I_sb = cpool.tile([128, 128], BF, tag="I")
            nc.gpsimd.dma_start(out=I_sb[:], in_=eye[:, :])
            ones_sb = cpool.tile([128, 1], BF, tag="ones")
            nc.gpsimd.dma_start(out=ones_sb[:], in_=ones[:, :])
            onesr_sb = cpool.tile([1, DH], BF, tag="onesr")
            nc.gpsimd.dma_start(out=onesr_sb[:], in_=onesr[:, :])
            nmd_sb = cpool.tile([128, 128], BF, tag="nmd")
            nc.gpsimd.dma_start(out=nmd_sb[:], in_=negm[:, :])
            wo_sb = cpool.tile([128, HPC * D], BF, tag="wo")
            nc.gpsimd.dma_start(out=wo_sb[:], in_=wo[:, :])

            # DVE warm-up reads: advance DVE's observed DMA-lane clocks so
            # later 3-AP TensorTensor ops need at most one sync wait (walrus
            # rejects TT with 2+ waits on this toolchain).
            dmy = wpool.tile([1, 16], BF, tag="dmy", bufs=1)
            for wsrc in (C_sb, S_sb):
                nc.vector.tensor_copy(dmy[:], wsrc[0:1, 0:16])

            # ---- persistent QKV / attention-output buffers ----
            # qT/kT/oT[h]: [DH, L]; v: [128(tok-in-tile), TT*E] tok-tile-major
            qT = [qkvpool.tile([128, L], BF, tag=f"q{h}", name=f"q{h}") for h in range(HPC)]
            kT = [qkvpool.tile([128, L], BF, tag=f"k{h}", name=f"k{h}") for h in range(HPC)]
            oT = [qkvpool.tile([128, L], BF, tag=f"o{h}", name=f"o{h}") for h in range(HPC)]
            v_sb = qkvpool.tile([128, TT * E], BF, tag="v")

            # ================= Phase A: QKV projection + RoPE =================
            with (
                tc.tile_pool(name="pa_ps", bufs=3, space="PSUM") as pa_ps,
                tc.tile_pool(name="pb_ps", bufs=2, space="PSUM") as pb_ps,
            ):
                def load_xc(c, nsplit):
                    # contiguous column range of xP; SBUF layout is the
                    # matmul-native xc[p, t*QC + q]
                    xc = xpool.tile([128, KT * QC], BF, tag="xc")
                    hq = KT * QC // nsplit
                    q0 = c * KT * QC
                    for s in range(nsplit):
                        nc.sync.dma_start(
                            out=xc[:, s * hq:(s + 1) * hq],
                            in_=xP[:, q0 + s * hq:q0 + (s + 1) * hq])
                    return xc

                # SP ring: x chunks only; weights stream on SWDGE in
                # parallel.
                xc_first = load_xc(0, 2)

                for c in range(NCH):  # 4 chunks of 512 tokens
                    l0 = c * QC
                    xc = xc_first if c == 0 else load_xc(c, 2)

                    # rope tail of group g is deferred one group so the PE
                    # never waits on the ACT evacuation of qa.
                    def rope_tail(g):
                        qa, dest = g
                        qb = pb_ps.tile([128, QC], F32, tag="qb")
                        nc.tensor.matmul(qb[:], A_sb[:], qa[:], start=True, stop=True)
                        t2 = wpool.tile([128, QC], BF, tag="t2", bufs=2)
                        nc.vector.tensor_mul(t2[:], qb[:], S_sb[:, l0:l0 + QC])
                        t1 = wpool.tile([128, QC], BF, tag="t1", bufs=2)
                        nc.vector.tensor_mul(t1[:], qa[:], C_sb[:, l0:l0 + QC])
                        nc.vector.tensor_add(dest[:, l0:l0 + QC], t1[:], t2[:])

                    pending = None
                    # q/k projections + rope -> qT/kT
                    for (wsb, dest) in ((wq_sb, qT), (wk_sb, kT)):
                        if c == 0:
                            # chunk 0 is DMA-latency bound: sweep k-tile
                            # BLOCKS (matching the wq DMA splits) across all
                            # heads so the PE consumes weight tiles in
                            # arrival order instead of stalling per head.
                            accs = [pa_ps.tile([128, QC], F32, tag="acc",
                                               bufs=4, name=f"acc{hh}")
                                    for hh in range(HPC)]
                            t0 = 0
                            for blk in (2, 2, 4, 8):
                                for h in range(HPC):
                                    for t in range(t0, t0 + blk):
                                        nc.tensor.matmul(
                                            accs[h][:],
                                            wsb[:, t * E + h * 128: t * E + (h + 1) * 128],
                                            xc[:, t * QC:(t + 1) * QC],
                                            start=(t == 0), stop=(t == KT - 1))
                                t0 += blk
                            for h in range(HPC):
                                qa = wpool.tile([128, QC], BF, tag="qa")
                                nc.scalar.activation(qa[:], accs[h][:],
                                                     mybir.ActivationFunctionType.Copy)
                                if pending is not None:
                                    rope_tail(pending)
                                pending = (qa, dest[h])
                            continue_heads = ()
                        else:
                            continue_heads = range(HPC)
                        for h in continue_heads:
                            acc = pa_ps.tile([128, QC], F32, tag="acc", bufs=4)
                            for t in range(KT):
                                nc.tensor.matmul(
                                    acc[:], wsb[:, t * E + h * 128: t * E + (h + 1) * 128],
                                    xc[:, t * QC:(t + 1) * QC],
                                    start=(t == 0), stop=(t == KT - 1))
                            qa = wpool.tile([128, QC], BF, tag="qa")
                            nc.scalar.activation(qa[:], acc[:],
                                                 mybir.ActivationFunctionType.Copy)
                            if pending is not None:
                                rope_tail(pending)
                            pending = (qa, dest[h])
                    # v projection (natural [token, feat] layout, free dim E=512)
                    for sub in range(4):
                        vacc = pa_ps.tile([128, E], F32, tag="acc", bufs=4)
                        for t in range(KT):
                            nc.tensor.matmul(
                                vacc[:], xc[:, t * QC + sub * 128: t * QC + (sub + 1) * 128],
                                wv_sb[:, t * E:(t + 1) * E], start=(t == 0), stop=(t == KT - 1))
                        i = 4 * c + sub  # global token tile index
                        nc.scalar.activation(v_sb[:, i * E:(i + 1) * E], vacc[:],
                                             mybir.ActivationFunctionType.Copy)
                        if sub == 0 and pending is not None:
                            rope_tail(pending)
                            pending = None

            # ================= Phase B: attention =================
            # Engine budget per head-group: the exp ACTIVATEs dominate the
            # ACT (~1ns/elem, no accel), so everything else is kept off it:
            #  - score k-tiles are emitted in PAIRS sharing one 2-bank PSUM
            #    tile; full pairs get ONE merged exp (halves the ACT fixed
            #    cost), diagonal pairs two truncated exps.
            #  - the softmax denominator accumulates IN PSUM via one cheap
            #    ones-matmul per k-tile (M=1, rides the PE's idle); no DVE
            #    chains.
            #  - otp is evacuated UNNORMALIZED (DVE copy) right after its
            #    last AV matmul; normalization is pipelined two groups
            #    behind (norm1: reciprocal+cast at i==1 of g+1; norm2:
            #    broadcast-MM + in-place scale at i==3 of g+2), so the PE
            #    never waits on the slow [1,QC] reciprocal.
            with (
                tc.tile_pool(name="sc_ps", bufs=2, space="PSUM") as sc_ps,
                tc.tile_pool(name="den_ps", bufs=2, space="PSUM") as den_ps,
                tc.tile_pool(name="ot_ps", bufs=2, space="PSUM") as ot_ps,
            ):
                def emit_pair(h, j, m):
                    """Scores+exp for k-tiles (2m, 2m+1) of q-chunk j, in one
                    [128, 2*QC] PSUM tile. Returns (ex2, (off0, off1))."""
                    sc2 = sc_ps.tile([128, 2 * QC], F32, tag="scp")
                    ex2 = expool.tile([128, 2 * QC], BF, tag="ex", bufs=3)
                    offs = []
                    for half, i in enumerate((2 * m, 2 * m + 1)):
                        r = i - 4 * j
                        off = 128 * r if r >= 0 else 0
                        base = half * QC
                        nc.tensor.matmul(sc2[:, base + off:base + QC],
                                         kT[h][:, i * 128:(i + 1) * 128],
                                         qT[h][:, j * QC + off:(j + 1) * QC],
                                         start=True, stop=(r < 0))
                        if r >= 0:  # strict-lower-triangle -1e9 on the diagonal
                            nc.tensor.matmul(sc2[:, base + off:base + off + 128],
                                             I_sb[:], nmd_sb[:], start=False, stop=True)
                        offs.append(off)
                    # one merged exp for full pairs only: evaluating exp over
                    # never-written PSUM (diag pairs' masked prefix) can
                    # produce inf/NaN and proved flaky
                    if offs[0] == 0 and offs[1] == 0:
                        nc.scalar.activation(ex2[:, offs[0]:], sc2[:, offs[0]:],
                                             mybir.ActivationFunctionType.Exp,
                                             scale=float(ISCALE))
                    else:
                        for half, off in enumerate(offs):
                            nc.scalar.activation(
                                ex2[:, half * QC + off:(half + 1) * QC],
                                sc2[:, half * QC + off:(half + 1) * QC],
                                mybir.ActivationFunctionType.Exp,
                                scale=float(ISCALE))
                    return (ex2, offs)

                def norm1(args):
                    # ones-MM collapses the DVE-accumulated exp-sum along
                    # partitions, then 1/den as exp(-ln(den)) on the ACT:
                    # the DVE reciprocal is 8 cycles/element on a single
                    # lane (3.3us for [1,QC]); Ln+Exp share one ACT table
                    # set and cost ~0.7us each.
                    h, j, esum = args
                    den = den_ps.tile([1, QC], F32, tag="den")
                    nc.tensor.matmul(den[:], ones_sb[:, :1], esum[:],
                                     start=True, stop=True)
                    lnd = wpool.tile([1, QC], F32, tag="recf", bufs=3)
                    nc.scalar.activation(lnd[:], den[:],
                                         mybir.ActivationFunctionType.Ln)
                    rec = wpool.tile([1, QC], BF, tag="rec", bufs=3)
                    nc.scalar.activation(rec[:], lnd[:],
                                         mybir.ActivationFunctionType.Exp,
                                         scale=-1.0)
                    return (h, j, rec)

                def norm2(args):
                    h, j, rec = args
                    # bcp time-shares the otp slots (freed by the previous
                    # group's evacuation)
                    bcp = ot_ps.tile([128, QC], F32, tag="otp")
                    nc.tensor.matmul(bcp[:], onesr_sb[:], rec[:], start=True, stop=True)
                    recb = wpool.tile([128, QC], BF, tag="recb", bufs=2)
                    nc.vector.tensor_copy(recb[:], bcp[:])
                    nc.vector.tensor_mul(oT[h][:, j * QC:(j + 1) * QC],
                                         oT[h][:, j * QC:(j + 1) * QC], recb[:])

                pend1 = None
                norm_q = []  # norm1 outputs awaiting norm2
                groups = [(h, j) for h in range(HPC) for j in range(NCH)]
                pre = None  # next group's first pair, emitted during this tail
                for gi, (h, j) in enumerate(groups):
                        nk = 4 * j + 4  # causal: k-tiles 0..4j+3
                        npair = nk // 2
                        otp = ot_ps.tile([128, QC], F32, tag="otp")
                        esum = wpool.tile([128, QC], BF, tag="esum", bufs=2)
                        pairq = [pre if pre is not None else emit_pair(h, j, 0)]
                        pre = None
                        if npair > 1:
                            pairq.append(emit_pair(h, j, 1))
                        for i in range(nk):
                            m, half = divmod(i, 2)
                            if half == 0 and m + 2 < npair:
                                pairq.append(emit_pair(h, j, m + 2))
                            if i == nk - 2 and gi + 1 < len(groups):
                                # cross-group prefetch: the next group's
                                # first exp is in flight before its AV needs
                                # it, killing the group-start refill bubble
                                hn, jn = groups[gi + 1]
                                pre = emit_pair(hn, jn, 0)
                            ex2, offs = pairq[m]
                            off = offs[half]
                            sl = slice(half * QC + off, (half + 1) * QC)
                            nc.tensor.matmul(otp[:, off:],
                                             v_sb[:, i * E + h * 128: i * E + (h + 1) * 128],
                                             ex2[:, sl], start=(i == 0), stop=(i == nk - 1))
                            # DVE denominator chain (i==0 is always full width)
                            if i == 0:
                                nc.vector.tensor_copy(esum[:], ex2[:, sl])
                            else:
                                nc.vector.tensor_add(esum[:, off:], esum[:, off:],
                                                     ex2[:, sl])
                        # Group tail, in this order: (1) unnormalized otp
                        # evacuation (prompt slot recycling), (2) oldest
                        # pending norm2 (its reciprocal is >=1.5 groups old;
                        # skipped after short j==0 groups where it would
                        # still be in flight), (3) previous group's norm1.
                        nc.vector.tensor_copy(oT[h][:, j * QC:(j + 1) * QC], otp[:])
                        # pop: every item >=2 groups old (catch-up after a
                        # j==0 deferral), plus the standard 1-group-old item
                        # except right after short j==0 groups
                        gcur = h * NCH + j
                        while norm_q and gcur - norm_q[0][0] >= 2:
                            norm2(norm_q.pop(0)[1])
                        if norm_q and j != 0 and gcur - norm_q[0][0] >= 1:
                            norm2(norm_q.pop(0)[1])
                        if pend1 is not None:
                            norm_q.append((gcur, norm1(pend1)))
                        pend1 = (h, j, esum)
                # final norm1; the remaining (last two) norm2s are emitted
                # inside phase C (their reciprocals are still in flight on
                # the ACT, and their oT chunks are not needed until tt>=8)
                norm_q.append((len(norm_q), norm1(pend1)))
                norm_q = [it for _, it in norm_q]
                assert len(norm_q) == 2, len(norm_q)

            # ================= Phase C: output projection =================
            with tc.tile_pool(name="y_ps", bufs=2, space="PSUM") as y_ps:
                def norm2_late(args):
                    h, j, rec = args
                    bcp = y_ps.tile([128, QC], F32, tag="bcp", bufs=1)
                    nc.tensor.matmul(bcp[:], onesr_sb[:], rec[:], start=True, stop=True)
                    recb = wpool.tile([128, QC], BF, tag="recb", bufs=2)
                    nc.vector.tensor_copy(recb[:], bcp[:])
                    nc.vector.tensor_mul(oT[h][:, j * QC:(j + 1) * QC],
                                         oT[h][:, j * QC:(j + 1) * QC], recb[:])

                for tt in range(TT):  # 16 token tiles
                    ysb = ypool.tile([128, D], BF, tag="ysb")
                    row0 = tt * 128
                    last = tt == TT - 1
                    for half in range(2):  # 2-bank PSUM tiles
                        yp = y_ps.tile([128, D // 2], F32, tag="yp")
                        h0 = half * (D // 2)
                        for n in range(2):  # 2 d-chunks per half
                            for h in range(HPC):
                                nc.tensor.matmul(
                                    yp[:, n * QC:(n + 1) * QC],
                                    oT[h][:, tt * 128:(tt + 1) * 128],
                                    wo_sb[:, h * D + h0 + n * QC: h * D + h0 + (n + 1) * QC],
                                    start=(h == 0), stop=(h == HPC - 1))
                        if last:  # split the final evac/stores to shorten the tail
                            for n in range(2):
                                nc.scalar.activation(
                                    ysb[:, h0 + n * QC:h0 + (n + 1) * QC],
                                    yp[:, n * QC:(n + 1) * QC],
                                    mybir.ActivationFunctionType.Copy)
                                nc.sync.dma_start(
                                    out=y[row0:row0 + 128, h0 + n * QC:h0 + (n + 1) * QC],
                                    in_=ysb[:, h0 + n * QC:h0 + (n + 1) * QC])
                        else:
                            nc.scalar.activation(ysb[:, h0:h0 + D // 2], yp[:],
                                                 mybir.ActivationFunctionType.Copy)
                    if not last:
                        nc.sync.dma_start(out=y[row0:row0 + 128, :], in_=ysb[:])
                    if tt < len(norm_q):
                        # the chunk must not be needed before this tt
                        assert norm_q[tt][1] * 4 > tt
                        norm2_late(norm_q[tt])
    return nc


def _prep_inputs(x, cos, sin, wq, wk, wv, wo):
    """Host-side sharding + layout prep. Returns in_maps for the 8 cores."""
    bf = ml_dtypes.bfloat16
    # per-batch partition-major activations, chunk-tile pre-tiled:
    # xP[p, (c*KT + t)*QC + q] = x[c*QC + q, t*128 + p]
    xPb = [np.ascontiguousarray(
        x[b].reshape(NCH, QC, KT, 128).transpose(3, 0, 2, 1)
            .reshape(128, L * KT)
    ).astype(bf) for b in range(B)]

    def wperm(wslice):
        # [E_out, D] nn.Linear weight slice -> wP[p, t*E + e] = w.T[t*128+p, e]
        wT = np.ascontiguousarray(wslice.T)                       # [D, E]
        return np.ascontiguousarray(
            wT.reshape(KT, 128, wT.shape[1]).transpose(1, 0, 2)
              .reshape(128, KT * wT.shape[1])).astype(bf)
    # RoPE tables in transposed pair-broadcast layout [DH, L]
    Ct = np.repeat(cos.T, 2, axis=0).astype(bf)                      # [128, L]
    St = np.repeat(sin.T, 2, axis=0).astype(bf)
    # pair swap/negate matrix A: qb[2i] = -qa[2i+1], qb[2i+1] = qa[2i]
    A = np.zeros((DH, DH), np.float32)
    for i in range(DH // 2):
        A[2 * i, 2 * i + 1] = -1.0
        A[2 * i + 1, 2 * i] = 1.0
    At = np.ascontiguousarray(A.T).astype(bf)
    eye = np.eye(DH, dtype=np.float32).astype(bf)
    ones = np.ones((DH, 1), np.float32).astype(bf)
    # strict lower triangle -1e9: masked where local q-col c < k-row k
    nm = np.where(np.arange(128)[None, :] < np.arange(128)[:, None],
                  np.float32(NEG), np.float32(0.0)).astype(bf)
    in_maps = []
    for c in range(NCORES):
        b = c // GROUPS
        sl = slice((c % GROUPS) * E, (c % GROUPS + 1) * E)
        woT = np.ascontiguousarray(wo[:, sl].T)                    # [E, D]
        woP = np.ascontiguousarray(
            woT.reshape(HPC, 128, D).transpose(1, 0, 2).reshape(128, HPC * D)
        ).astype(bf)
        in_maps.append({
            "xP": xPb[b],
            "wqP": wperm(wq[sl, :]),
            "wkP": wperm(wk[sl, :]),
            "wvP": wperm(wv[sl, :]),
            "woP": woP,
            "Ct": Ct, "St": St, "At": At, "eye": eye, "ones": ones,
            "onesr": np.ones((1, DH), np.float32).astype(bf), "negm": nm,
        })
    return in_maps


_NC_CACHE = {}


def run(x, cos, sin, wq, wk, wv, wo, trace=False):
    if "nc" not in _NC_CACHE:
        _NC_CACHE["nc"] = build_nc()
    nc = _NC_CACHE["nc"]
    in_maps = _prep_inputs(x, cos, sin, wq, wk, wv, wo)
    res = run_bass_kernel_spmd(nc, in_maps, core_ids=list(range(NCORES)), trace=trace)
    parts = [r["y"].astype(np.float32) for r in res.results]
    y = np.stack([sum(parts[b * GROUPS:(b + 1) * GROUPS]) for b in range(B)])
    return y, res


def kernel(x, mask, cos, sin, wq, wk, wv, wo):
    x = np.asarray(x, np.float32)
    y, _ = run(x, np.asarray(cos, np.float32), np.asarray(sin, np.float32),
               np.asarray(wq, np.float32), np.asarray(wk, np.float32),
               np.asarray(wv, np.float32), np.asarray(wo, np.float32))
    return y.astype(np.float32)

